# revision 2
# baseline (speedup 1.0000x reference)
import sys

if "/opt/trn_rl_repo" not in sys.path:
    sys.path.insert(0, "/opt/trn_rl_repo")

import os
import numpy as np
import ml_dtypes

_DBG_LAYERS = int(os.environ.get("GCN_DBG_LAYERS", "2"))
_DBG_COLL = int(os.environ.get("GCN_DBG_COLL", "1"))
_DBG_GROUPS = int(os.environ.get("GCN_DBG_GROUPS", "7"))
GCHUNK = int(os.environ.get("GCN_GCHUNK", "16"))   # tiles per dma_gather call
GBUFS = int(os.environ.get("GCN_GBUFS", "16"))     # gather chunk buffers
_MSPLIT = int(os.environ.get("GCN_MSPLIT", "1000000"))  # every Nth M-build on gpsimd

import concourse.bass as bass
import concourse.bacc as bacc
import concourse.mybir as mybir
import concourse.tile as tile
from concourse.bass_utils import run_bass_kernel_spmd

N = 100000
E = 1600000
IN = 128
HID = 128
OUT = 64

NCORES = 8
PCORE = N // NCORES            # 12500 nodes per core
BLK = 128                      # dst nodes per block (psum tile width)
NBLK = 98                      # blocks per core (98*128 = 12544 >= 12500)
GRP = 14                       # blocks per group
NGRP = 7                       # groups per core
CHUNK = GRP * BLK              # 1792 rows per allgather chunk
TROWS = NCORES * NBLK * BLK    # 100352 table rows (padded, permuted)
NWIN = 7                       # source windows == allgather chunks
WIN = TROWS // NWIN            # 14336 rows per source window (< 32768 for int16 idx)

BF16 = mybir.dt.bfloat16
F32 = mybir.dt.float32
I16 = mybir.dt.int16

_cache = {}


def _node_pos(n):
    """Table position of node id n under the group-interleaved permutation."""
    c = n // PCORE
    r = n - c * PCORE
    g = r // CHUNK
    wi = r - g * CHUNK
    return (g * NCORES + c) * CHUNK + wi


def _preprocess(x, edge_index):
    src = edge_index[0].astype(np.int64)
    dst = edge_index[1].astype(np.int64)
    loops = np.arange(N, dtype=np.int64)
    s = np.concatenate([src, loops])
    d = np.concatenate([dst, loops])

    deg = np.bincount(d, minlength=N).astype(np.float64)
    dinv = (1.0 / np.sqrt(deg)).astype(np.float32)
    norm = (dinv[s] * dinv[d]).astype(np.float32)

    pos = _node_pos(np.arange(N, dtype=np.int64))
    ps = pos[s]
    w_e = ps // WIN
    wloc = (ps - w_e * WIN).astype(np.int64)

    core_e = d // PCORE
    r = d - core_e * PCORE
    b_e = r // BLK
    dloc = (r - b_e * BLK).astype(np.float32)
    g_e = b_e // GRP

    # sort edges by (core, group, window, block)
    key = ((core_e * NGRP + g_e) * NWIN + w_e) * NBLK + b_e
    order = np.argsort(key, kind="stable")
    key_s = key[order]
    wloc_s = wloc[order]
    dloc_s = dloc[order]
    norm_s = norm[order]

    nruns = NCORES * NGRP * NWIN * NBLK
    cnt = np.bincount(key_s, minlength=nruns).reshape(NCORES, NGRP, NWIN, NBLK)
    # uniform (across cores) tiles per (g, w, b); b is global block id 0..NBLK-1
    tiles = -(-cnt.max(axis=0) // BLK)  # ceil div, shape [NGRP, NWIN, NBLK]

    # per-core flat padded arrays in (g, w, block-within-group) order
    run_starts = np.zeros(nruns + 1, np.int64)
    np.cumsum(cnt.reshape(-1), out=run_starts[1:])

    tot_tiles = 0
    for g in range(NGRP):
        for w in range(NWIN):
            for j in range(GRP):
                tot_tiles += int(tiles[g, w, g * GRP + j])
    TOT = tot_tiles * BLK

    idx_w = np.zeros((NCORES, 128, TOT // 16), np.int16)
    dst_w = np.full((NCORES, 128, tot_tiles), -1.0, np.float32)
    nrm_w = np.zeros((NCORES, 128, tot_tiles), np.float32)

    flat_i = np.zeros(TOT, np.int64)
    flat_d = np.empty(TOT, np.float32)
    flat_n = np.zeros(TOT, np.float32)
    for c in range(NCORES):
        flat_i[:] = 0
        flat_d[:] = -1.0
        flat_n[:] = 0.0
        off = 0
        for g in range(NGRP):
            for w in range(NWIN):
                for j in range(GRP):
                    b = g * GRP + j
                    t = int(tiles[g, w, b])
                    if t == 0:
                        continue
                    rid = ((c * NGRP + g) * NWIN + w) * NBLK + b
                    a0, a1 = run_starts[rid], run_starts[rid + 1]
                    n_e = a1 - a0
                    flat_i[off:off + n_e] = wloc_s[a0:a1]
                    flat_d[off:off + n_e] = dloc_s[a0:a1]
                    flat_n[off:off + n_e] = norm_s[a0:a1]
                    off += t * BLK
        assert off == TOT
        w16 = flat_i.reshape(-1, 16).T.astype(np.int16)   # [16, TOT/16]
        idx_w[c] = np.tile(w16, (8, 1))
        dst_w[c] = flat_d.reshape(-1, 128).T
        nrm_w[c] = flat_n.reshape(-1, 128).T

    # permuted, padded bf16 source table
    xb = np.zeros((TROWS, IN), ml_dtypes.bfloat16)
    xb[pos] = x.astype(ml_dtypes.bfloat16)

    sched = tuple(
        tuple(tuple(int(tiles[g, w, g * GRP + j]) for j in range(GRP))
              for w in range(NWIN))
        for g in range(NGRP)
    )
    return sched, xb, idx_w, dst_w, nrm_w


def _build(sched, zero_bias=False):
    """Build the 8-core SPMD Bass program for a given tile schedule."""
    tot_tiles = sum(t for g in sched for w in g for t in w)

    nc = bacc.Bacc("TRN2", target_bir_lowering=False, debug=False,
                   enable_asserts=False, num_devices=NCORES)

    xb_d = nc.dram_tensor("xb", [TROWS, IN], BF16, kind="ExternalInput")
    idx_d = nc.dram_tensor("idxw", [128, tot_tiles * 8], I16, kind="ExternalInput")
    dst_d = nc.dram_tensor("dstw", [128, tot_tiles], F32, kind="ExternalInput")
    nrm_d = nc.dram_tensor("nrmw", [128, tot_tiles], F32, kind="ExternalInput")
    iota_d = nc.dram_tensor("iota", [128, BLK], BF16, kind="ExternalInput")
    w1_d = nc.dram_tensor("w1b", [IN, HID], BF16, kind="ExternalInput")
    w2_d = nc.dram_tensor("w2b", [HID, OUT], BF16, kind="ExternalInput")
    b1_d = nc.dram_tensor("b1t", [128, HID], F32, kind="ExternalInput")
    b2_d = nc.dram_tensor("b2t", [128, OUT], F32, kind="ExternalInput")
    out_d = nc.dram_tensor("out", [NBLK * BLK, OUT], F32, kind="ExternalOutput")

    with tile.TileContext(nc) as tc:
        with tc.tile_pool(name="const", bufs=1) as cpool, \
             tc.tile_pool(name="meta", bufs=1) as mpool_meta, \
             tc.tile_pool(name="idx", bufs=4) as ipool, \
             tc.tile_pool(name="gat", bufs=GBUFS) as gpool, \
             tc.tile_pool(name="m", bufs=8) as mpool, \
             tc.tile_pool(name="agg", bufs=3) as apool, \
             tc.tile_pool(name="post", bufs=3) as ppool, \
             tc.tile_pool(name="psum_g", bufs=4, space="PSUM") as psg, \
             tc.tile_pool(name="psum_t", bufs=2, space="PSUM") as pst, \
             tc.tile_pool(name="dram", bufs=2, space="DRAM") as dpool:

            iota_t = cpool.tile([128, BLK], BF16)
            w1_t = cpool.tile([IN, HID], BF16)
            w2_t = cpool.tile([HID, OUT], BF16)
            b1_t = cpool.tile([128, HID], F32)
            b2_t = cpool.tile([128, OUT], F32)
            nc.sync.dma_start(iota_t[:], iota_d[:])
            nc.sync.dma_start(w1_t[:], w1_d[:])
            nc.sync.dma_start(w2_t[:], w2_d[:])
            nc.sync.dma_start(b1_t[:], b1_d[:])
            nc.sync.dma_start(b2_t[:], b2_d[:])

            dst_t = mpool_meta.tile([128, tot_tiles], F32)
            nrm_t = mpool_meta.tile([128, tot_tiles], F32)
            nc.sync.dma_start(dst_t[:], dst_d[:])
            nc.sync.dma_start(nrm_t[:], nrm_d[:])

            h1_parts = [
                dpool.tile([WIN, HID], BF16, bufs=1, addr_space="Shared",
                           name=f"h1p{w}", tag=f"h1p{w}")
                for w in range(NWIN)
            ]

            for layer in range(_DBG_LAYERS):
                w_t = w1_t if layer == 0 else w2_t
                b_t = b1_t if layer == 0 else b2_t
                ow = HID if layer == 0 else OUT
                t_base = 0
                for g in range(min(NGRP, _DBG_GROUPS)):
                    toff = {}
                    tb = t_base
                    for w in range(NWIN):
                        for j in range(GRP):
                            t = sched[g][w][j]
                            if t == 0:
                                continue
                            toff[(w, j)] = tb
                            tb += t
                    if layer == 0:
                        h1own = dpool.tile([CHUNK, HID], BF16, tag="h1own")
                    # gather this group's edges: per (subgroup of SUB blocks, window)
                    gat_of = {}   # (w, sub) -> (tile, start_tile)
                    SUB = 7
                    for sub in range(0, GRP, SUB):
                        for w in range(NWIN):
                            js = [j for j in range(sub, min(sub + SUB, GRP))
                                  if sched[g][w][j] > 0]
                            if not js:
                                continue
                            cs = toff[(w, js[0])]
                            nt = sum(sched[g][w][j] for j in js)
                            assert nt <= 63, f"gather too large: {nt} tiles"
                            tab_ap = (xb_d[w * WIN:(w + 1) * WIN, :] if layer == 0
                                      else h1_parts[w][:])
                            idx_t = ipool.tile([128, nt * 8], I16, tag="idx")
                            nc.sync.dma_start(idx_t[:], idx_d[:, cs * 8:(cs + nt) * 8])
                            gat = gpool.tile([128, nt, IN], BF16, tag="gat")
                            nc.gpsimd.dma_gather(
                                gat[:], tab_ap, idx_t[:],
                                nt * BLK, nt * BLK, IN,
                                single_packet=False,
                            )
                            gat_of[(w, sub // SUB)] = (gat, cs)
                        # per block: consecutive accumulation into one psum bank
                        for j in range(sub, min(sub + SUB, GRP)):
                            tl = []
                            for w in range(NWIN):
                                t = sched[g][w][j]
                                if t:
                                    tj = toff[(w, j)]
                                    tl.extend((w, tt) for tt in range(tj, tj + t))
                            pj = psg.tile([128, BLK], F32, tag="pj")
                            for i, (w, tt) in enumerate(tl):
                                m = mpool.tile([128, BLK], BF16, tag="m")
                                meng = nc.vector if (tt % _MSPLIT) else nc.gpsimd
                                meng.tensor_scalar(
                                    m[:], iota_t[:],
                                    dst_t[:, tt:tt + 1], nrm_t[:, tt:tt + 1],
                                    mybir.AluOpType.is_equal,
                                    mybir.AluOpType.mult,
                                )
                                gat, cs = gat_of[(w, j // SUB)]
                                nc.tensor.matmul(
                                    pj[:], lhsT=gat[:, tt - cs, :],
                                    rhs=m[:],
                                    start=(i == 0), stop=(i == len(tl) - 1),
                                )
                            aggs = apool.tile([128, BLK], BF16, tag="agg")
                            nc.scalar.activation(aggs[:], pj[:],
                                                 mybir.ActivationFunctionType.Copy)
                            ptr = pst.tile([128, ow], F32, tag="ptr")
                            nc.tensor.matmul(ptr[:], lhsT=aggs[:], rhs=w_t[:],
                                             start=True, stop=True)
                            if layer == 0:
                                if zero_bias:
                                    hb = ptr
                                else:
                                    hb = ppool.tile([128, HID], F32, tag="hb")
                                    nc.vector.tensor_add(hb[:], ptr[:], b_t[:])
                                h1b = ppool.tile([128, HID], BF16, tag="h1b")
                                nc.scalar.activation(h1b[:], hb[:],
                                                     mybir.ActivationFunctionType.Relu)
                                nc.sync.dma_start(h1own[j * BLK:(j + 1) * BLK, :], h1b[:])
                                if _DBG_LAYERS == 1:
                                    nc.sync.dma_start(
                                        out_d[(g * GRP + j) * BLK:(g * GRP + j + 1) * BLK, :],
                                        hb[:, :OUT])
                            else:
                                ob = ppool.tile([128, OUT], F32, tag="ob")
                                if zero_bias:
                                    nc.vector.tensor_copy(ob[:], ptr[:])
                                else:
                                    nc.vector.tensor_add(ob[:], ptr[:], b_t[:])
                                nc.sync.dma_start(
                                    out_d[(g * GRP + j) * BLK:(g * GRP + j + 1) * BLK, :],
                                    ob[:])
                    if layer == 0 and _DBG_COLL:
                        nc.gpsimd.collective_compute(
                            "AllGather",
                            mybir.AluOpType.bypass,
                            ins=[h1own.opt()],
                            outs=[h1_parts[g].opt()],
                            replica_groups=[list(range(NCORES))],
                        )
                    t_base = tb

    nc.compile()
    return nc


def kernel(x, edge_index, W1, b1, W2, b2):
    sched, xb, idx_w, dst_w, nrm_w = _preprocess(np.asarray(x), np.asarray(edge_index))

    zero_bias = (not np.any(np.asarray(b1))) and (not np.any(np.asarray(b2)))
    key = (sched, zero_bias)
    if key not in _cache:
        _cache[key] = _build(sched, zero_bias)
    nc = _cache[key]

    iota = np.tile(np.arange(BLK, dtype=np.float32), (128, 1)).astype(ml_dtypes.bfloat16)
    w1b = np.asarray(W1).astype(ml_dtypes.bfloat16)
    w2b = np.asarray(W2).astype(ml_dtypes.bfloat16)
    b1t = np.tile(np.asarray(b1, dtype=np.float32), (128, 1))
    b2t = np.tile(np.asarray(b2, dtype=np.float32), (128, 1))

    in_maps = []
    for c in range(NCORES):
        in_maps.append({
            "xb": xb, "idxw": idx_w[c], "dstw": dst_w[c], "nrmw": nrm_w[c],
            "iota": iota, "w1b": w1b, "w2b": w2b, "b1t": b1t, "b2t": b2t,
        })
    res = run_bass_kernel_spmd(nc, in_maps, core_ids=list(range(NCORES)),
                               trace=bool(int(os.environ.get("GCN_TRACE", "0"))),
                               tmpdir=os.environ.get("GCN_TMPDIR") or None)
    if res.exec_time_ns is not None:
        print(f"HW exec time: {res.exec_time_ns} ns")
        kernel.last_exec_ns = res.exec_time_ns

    out = np.empty((N, OUT), np.float32)
    for c in range(NCORES):
        out[c * PCORE:(c + 1) * PCORE] = res.results[c]["out"][:PCORE]
    return out

